# revision 5
# baseline (speedup 1.0000x reference)
"""Trainium2 Bass kernel for nn_ActorAttention (dense_transformer) — v2.

Math (from the reference): only the cls-token attention output is used and
the cls token is batch-constant, so q0/k0/v0 and the cls-cls score are
host-precomputed constants.  Per batch row:
    feats_j = relu(obs[a_j:b_j] @ Wenc_j + b_j)          j=0..4  [128]
    k_j     = relu(Wk_red^T feats_j)                      [64 used dims]
    E_j     = exp(q0_expanded . k_j)     (expanded to 128 rows, 16 per head)
    den     = e0 + sum_j mean16(E_j)                      [8]
    numy    = e0*v0 + sum_j E_j * relu(Wv^T feats_j)      [128]
    x       = relu(Wo^T (numy * (1/den)_expanded))        [128]
    m       = head_W^T x                                  [5]
    policy  = softmax(m - log(-log u))                    [5]

v2 layout/speed choices vs v1:
  - obs is transposed on the host; the per-tile load is one contiguous DMA
    that also replicates obsT into 4 partition groups (stride-0 source), so
    the 5 encoder matmuls run 4-way row-tiled (tile_position concurrency).
  - k projection is col-tiled: two tokens share one matmul slot pair-wise
    and one [128,N] relu covers two tokens.
  - scores row-tile in pairs (low/high k halves).
  - the attention combine accumulates in bf16 (tolerance is 2e-2).
  - the gumbel head transposes exp(m) with 4 tiny PE transposes and runs the
    softmax tail on [128, 4, 5] natural-layout tiles (free size 20).
  - all weights ship in ONE fused const DMA (kills the startup DMA chain).
"""

import numpy as np

F_DIM = 128
H = 8
D = 16
OBS_SLICES = [(0, 4), (4, 11), (11, 18), (18, 22), (22, 26)]
NCORES = 8
BTOT = 65536
BC = BTOT // NCORES  # 8192 rows per core
N = 512              # batch columns per tile

# fused const layout: name -> (rows, cols); column offsets assigned in order
_CONST_SHAPES = [
    ("wenc4", (128, 5 * F_DIM)),
    ("wkred2", (128, 256)),
    ("q0big2", (128, F_DIM)),
    ("onesda16", (128, H)),
    ("e0row", (1, H)),
    ("onesrow", (1, N)),
    ("bmatf", (8, F_DIM)),
    ("w0c", (8, F_DIM)),
    ("wv", (128, F_DIM)),
    ("wo", (128, F_DIM)),
    ("headw8", (128, 8)),
    ("id8", (8, 8)),
]
CONST_COLS = sum(c for _, (_, c) in _CONST_SHAPES)


def _const_offsets():
    off = {}
    o = 0
    for name, (r, c) in _CONST_SHAPES:
        off[name] = (o, r, c)
        o += c
    return off


def _consts_from_weights(inp):
    """Host-side weight preprocessing (all tiny, <1MB total)."""
    f32 = np.float32
    cls = np.asarray(inp["cls_token"], f32)
    q0 = np.maximum(cls @ np.asarray(inp["Wq"], f32) + np.asarray(inp["bq"], f32), 0)
    k0 = np.maximum(cls @ np.asarray(inp["Wk"], f32) + np.asarray(inp["bk"], f32), 0)
    v0 = np.maximum(cls @ np.asarray(inp["Wv"], f32) + np.asarray(inp["bv"], f32), 0)
    s0 = (q0.reshape(H, D) * k0.reshape(H, D)).sum(-1) / 4.0
    e0 = np.exp(s0).astype(f32)                                   # [8]
    numy0 = (e0[:, None] * v0.reshape(H, D)).reshape(F_DIM, 1)    # [128,1]

    c = {}
    # encoder weights: token j at row group (j%4)*32, col block j*128.
    # rows 32g+0..25 are obs features, row 32g+26 is the bias (ones row in
    # obsT4).  token 4 shares row group 0 with token 0 (different cols).
    wenc4 = np.zeros((128, 5 * F_DIM), f32)
    for j, (a, b) in enumerate(OBS_SLICES):
        g = (j % 4) * 32
        wenc4[g + a:g + b, j * F_DIM:(j + 1) * F_DIM] = np.asarray(
            inp[f"enc_W{j}"], f32)
        wenc4[g + 26, j * F_DIM:(j + 1) * F_DIM] = np.asarray(
            inp[f"enc_b{j}"], f32)
    c["wenc4"] = wenc4

    # reduced k projection: only dims where q0 != 0 matter for the scores
    kidx = np.nonzero(q0)[0]
    z = len(kidx)
    assert z <= 64, f"q0 nonzeros {z} > 64; packing assumption broken"
    wkred2 = np.zeros((128, 256), f32)
    wkred2[:F_DIM, :z] = np.asarray(inp["Wk"], f32)[:, kidx]          # L
    wkred2[:F_DIM, 128 + 64:128 + 64 + z] = np.asarray(
        inp["Wk"], f32)[:, kidx]                                       # H
    c["wkred2"] = wkred2

    # expanded scores: every output dim hd of head h carries head h's q0 row,
    # replicated at partitions 0:z and 64:64+z (for the two k halves)
    q0big2 = np.zeros((128, F_DIM), f32)
    for i, hd in enumerate(kidx):
        h = hd // D
        q0big2[i, h * D:(h + 1) * D] = q0[hd] / 4.0
        q0big2[64 + i, h * D:(h + 1) * D] = q0[hd] / 4.0
    c["q0big2"] = q0big2
    c["_z"] = int(z)

    onesda16 = np.zeros((128, H), f32)
    for hd in range(F_DIM):
        onesda16[hd, hd // D] = 1.0 / D
    c["onesda16"] = onesda16
    c["e0row"] = e0.reshape(1, H)
    c["onesrow"] = np.ones((1, N), f32)
    bmatf = np.zeros((H, F_DIM), f32)
    for h in range(H):
        bmatf[h, h * D:(h + 1) * D] = 1.0
    c["bmatf"] = bmatf
    c["wo"] = np.asarray(inp["Wo"], f32)
    c["w0c"] = (bmatf * numy0.reshape(1, F_DIM)) @ c["wo"]
    c["wv"] = np.asarray(inp["Wv"], f32)
    c["headw8"] = np.pad(np.asarray(inp["head_W"], f32), ((0, 0), (0, 3)))
    c["id8"] = np.eye(8, dtype=f32)

    # pack into the fused const block
    off = _const_offsets()
    blk = np.zeros((128, CONST_COLS), f32)
    for name, (o, r, cols) in off.items():
        blk[:r, o:o + cols] = c[name]
    return blk, c["_z"]


def _prep_core_inputs(obs, u, core):
    """Per-core host-side resharding: transposed obs + plain u."""
    f32 = np.float32
    lo = core * BC
    obsT = np.ascontiguousarray(np.concatenate(
        [np.asarray(obs[lo:lo + BC], f32).T,
         np.ones((1, BC), f32)]))                  # [27, BC]: row 26 = ones
    uu = np.ascontiguousarray(np.asarray(u[lo:lo + BC], f32))
    return obsT, uu


def build_program(bc=BC, repeat=1, z=61):
    """Single-core Bass/Tile program (SPMD across 8 cores)."""
    import contextlib
    from collections import deque

    import concourse.bass as bass
    import concourse.tile as tile
    from concourse import bacc, mybir

    f32 = mybir.dt.float32
    f32r = mybir.dt.float32r
    bf16 = mybir.dt.bfloat16
    ACT = mybir.ActivationFunctionType
    ALU = mybir.AluOpType
    AX = mybir.AxisListType

    nt = bc // N
    nc = bacc.Bacc(num_devices=NCORES)

    obsT_d = nc.dram_tensor("obsT", [27, bc + (repeat if repeat > 1 else 0)],
                            f32r, kind="ExternalInput").ap()
    u_d = nc.dram_tensor("u", [bc, 5], f32, kind="ExternalInput").ap()
    cblk1_d = nc.dram_tensor("cblk1", [128, 5 * F_DIM], f32r,
                             kind="ExternalInput").ap()
    cblk2_d = nc.dram_tensor("cblk2", [128, CONST_COLS - 5 * F_DIM], f32r,
                             kind="ExternalInput").ap()
    out_d = nc.dram_tensor("out", [bc, 5], f32, kind="ExternalOutput").ap()

    off = _const_offsets()

    with tile.TileContext(nc) as tc:
        with (
            tc.tile_pool(name="singles", bufs=1) as singles,
            tc.tile_pool(name="pfe", bufs=4) as pfe,
            tc.tile_pool(name="pks", bufs=3) as pks,
            tc.tile_pool(name="pE", bufs=4) as pE,
            tc.tile_pool(name="pT", bufs=3) as pT,
            tc.tile_pool(name="pacc", bufs=3) as pacc,
            tc.tile_pool(name="ptail", bufs=3) as ptail,
            tc.tile_pool(name="pfr", bufs=2, space="PSUM") as pfr,
            tc.tile_pool(name="pbk", bufs=1, space="PSUM") as pbk,
            tc.tile_pool(name="pss", bufs=2, space="PSUM") as pss,
        ):
            # single-tag pool allocators (each tag gets its own buf ring)
            cnt = {"fr": 0, "bk": 0, "sm": 0}

            def alloc_fr():
                cnt["fr"] += 1
                return pfr.tile([F_DIM, 2, N], f32, tag="fr",
                                name=f"fr_{cnt['fr']}")

            def alloc_bk():
                cnt["bk"] += 1
                return pbk.tile([F_DIM, 2, N], f32, tag="bk",
                                name=f"bk_{cnt['bk']}")

            def alloc_sm():
                cnt["sm"] += 1
                return pss.tile([F_DIM, N], f32, tag="sm",
                                name=f"sm_{cnt['sm']}")

            cblk = singles.tile([128, CONST_COLS], f32r, tag="cblk")
            nc.scalar.dma_start(out=cblk[:, 0:5 * F_DIM], in_=cblk1_d)
            nc.scalar.dma_start(out=cblk[:, 5 * F_DIM:], in_=cblk2_d)

            # hoisted gumbel-noise preprocessing: wrec = 1/ln(u) for all rows
            nj_all = bc // 128
            u_all = singles.tile([128, nj_all, 5], f32, tag="u_all")
            nc.scalar.dma_start(
                out=u_all, in_=u_d.rearrange("(j p) c -> p j c", p=128))
            l1_all = singles.tile([128, nj_all, 5], f32, tag="l1_all")
            nc.scalar.activation(l1_all, u_all, ACT.Ln)
            wrec_all = singles.tile([128, nj_all, 5], f32, tag="wrec_all")
            nc.vector.reciprocal_approx_fast(
                out=wrec_all.rearrange("p j c -> p (j c)"),
                in_=l1_all.rearrange("p j c -> p (j c)"))

            def cs(name):
                o, r, c = off[name]
                return cblk[0:r, o:o + c]

            wenc4 = cs("wenc4")
            wkredL = cs("wkred2")[:, 0:128]
            wkredH = cs("wkred2")[:, 128:256]
            q0big2 = cs("q0big2")
            onesda16 = cs("onesda16")
            e0row = cs("e0row")
            onesrow = cs("onesrow")
            bmatf = cs("bmatf")
            w0c = cs("w0c")
            wv = cs("wv")
            wo = cs("wo")
            headw8 = cs("headw8")
            id8 = cs("id8")

            # obsT4 ring: rows 32g+0..25 = obs features (4 replicas), row
            # 32g+26 = ones (bias row, preset once -- the DMA never touches it)
            obsT4s = [singles.tile([128, N], f32r, tag=f"obsT4_{i}",
                                   name=f"obsT4_{i}") for i in range(3)]

            def emit_front(t):
                base = t * N
                obsT4 = obsT4s[t % 3]
                rowlen = bc + (repeat if repeat > 1 else 0)
                for r4 in range(4):
                    osrc = bass.AP(tensor=obsT_d.tensor, offset=base,
                                   ap=[[rowlen, 27], [1, N]])
                    nc.sync.dma_start(out=obsT4[32 * r4:32 * r4 + 27, :],
                                      in_=osrc)

                feats = pfe.tile([F_DIM, 5, N], f32r, tag="feats")
                k_sb = pks.tile([F_DIM, 3, N], f32r, tag="k_sb")
                E_sb = pE.tile([F_DIM, 5, N], f32r, tag="E_sb")

                # encoder wave: tokens 0-3 row-tiled across the 4 groups
                pA = alloc_fr()
                pB = alloc_fr()
                for j in range(4):
                    g = 32 * j
                    dst = (pA, pB)[j // 2][:, j % 2, :]
                    nc.tensor.matmul(
                        dst, wenc4[g:g + 27, j * F_DIM:(j + 1) * F_DIM],
                        obsT4[g:g + 27, :], start=True, stop=True,
                        tile_position=(g, 0))
                nc.scalar.activation(feats[:, 0:2, :], pA, ACT.Relu)
                nc.vector.tensor_scalar_max(
                    out=feats[:, 2:4, :], in0=pB, scalar1=0.0)
                # token 4 (row group 0 again)
                pC = alloc_fr()
                nc.tensor.matmul(pC[:, 0, :], wenc4[0:27, 4 * F_DIM:5 * F_DIM],
                                 obsT4[0:27, :], start=True, stop=True)
                nc.scalar.activation(feats[:, 4, :], pC[:, 0, :], ACT.Relu)

                # k projection, col-tiled: tokens (2j, 2j+1) share a bank
                pkk = alloc_fr()
                nc.tensor.matmul(pkk[:, 0, :], wkredL, feats[:, 0, :],
                                 start=True, stop=False)
                nc.tensor.matmul(pkk[:, 0, :], wkredH, feats[:, 1, :],
                                 start=False, stop=True)
                nc.tensor.matmul(pkk[:, 1, :], wkredL, feats[:, 2, :],
                                 start=True, stop=False)
                nc.tensor.matmul(pkk[:, 1, :], wkredH, feats[:, 3, :],
                                 start=False, stop=True)
                nc.scalar.activation(k_sb[:, 0:2, :], pkk, ACT.Relu)
                pk3 = alloc_sm()
                nc.tensor.matmul(pk3, wkredH, feats[:, 4, :],
                                 start=True, stop=True)
                nc.vector.tensor_scalar_max(
                    out=k_sb[:, 2, :], in0=pk3, scalar1=0.0)

                # scores: j4 first (into pC's free slot), then row-tiled pairs
                nc.tensor.matmul(pC[:, 1, :], q0big2[64:64 + z, :],
                                 k_sb[64:64 + z, 2, :], start=True, stop=True)
                nc.scalar.activation(E_sb[:, 4, :], pC[:, 1, :], ACT.Exp)
                ps1 = alloc_fr()
                nc.tensor.matmul(ps1[:, 0, :], q0big2[0:z, :],
                                 k_sb[0:z, 0, :], start=True, stop=True)
                nc.tensor.matmul(ps1[:, 1, :], q0big2[64:64 + z, :],
                                 k_sb[64:64 + z, 0, :], start=True, stop=True)
                nc.scalar.activation(E_sb[:, 0:2, :], ps1, ACT.Exp)
                ps2 = alloc_fr()
                nc.tensor.matmul(ps2[:, 0, :], q0big2[0:z, :],
                                 k_sb[0:z, 1, :], start=True, stop=True)
                nc.tensor.matmul(ps2[:, 1, :], q0big2[64:64 + z, :],
                                 k_sb[64:64 + z, 1, :], start=True, stop=True)
                nc.scalar.activation(E_sb[:, 2:4, :], ps2, ACT.Exp)
                return feats, E_sb

            def emit_back(t, feats, E_sb):
                base = t * N
                # den
                den = alloc_sm()[0:H, :]
                nc.tensor.matmul(den, e0row, onesrow, start=True, stop=False)
                for j in range(5):
                    nc.tensor.matmul(den, onesda16, E_sb[:, j, :],
                                     start=False, stop=(j == 4))
                r_sb = pacc.tile([H, N], f32r, tag="r_sb")
                with nc.allow_low_precision(reason="f32r == f32 bits; rounding for matmul moving operand"):
                    nc.vector.reciprocal(out=r_sb, in_=den)
                Rps = alloc_sm()
                nc.tensor.matmul(Rps, bmatf, r_sb, start=True, stop=True)

                # v + combine (bf16 accumulation)
                T = pT.tile([F_DIM, 5, N], bf16, tag="T")
                pv1 = alloc_bk()
                for j in (0, 1):
                    nc.tensor.matmul(pv1[:, j, :], wv, feats[:, j, :],
                                     start=True, stop=True)
                nc.vector.scalar_tensor_tensor(
                    out=T[:, 0:2, :], in0=pv1, scalar=0.0,
                    in1=E_sb[:, 0:2, :].bitcast(f32),
                    op0=ALU.max, op1=ALU.mult)
                pv2 = alloc_bk()
                for j in (2, 3):
                    nc.tensor.matmul(pv2[:, j - 2, :], wv, feats[:, j, :],
                                     start=True, stop=True)
                nc.vector.scalar_tensor_tensor(
                    out=T[:, 2:4, :], in0=pv2, scalar=0.0,
                    in1=E_sb[:, 2:4, :].bitcast(f32),
                    op0=ALU.max, op1=ALU.mult)
                pv3 = alloc_bk()
                nc.tensor.matmul(pv3[:, 0, :], wv, feats[:, 4, :],
                                 start=True, stop=True)
                nc.vector.scalar_tensor_tensor(
                    out=T[:, 4, :], in0=pv3[:, 0, :], scalar=0.0,
                    in1=E_sb[:, 4, :].bitcast(f32),
                    op0=ALU.max, op1=ALU.mult)

                t01 = pacc.tile([F_DIM, N], bf16, tag="t01")
                nc.vector.tensor_tensor(out=t01, in0=T[:, 0, :],
                                        in1=T[:, 1, :], op=ALU.add)
                t23 = pacc.tile([F_DIM, N], bf16, tag="t23")
                nc.gpsimd.tensor_tensor(out=t23, in0=T[:, 2, :],
                                        in1=T[:, 3, :], op=ALU.add)
                a2 = pacc.tile([F_DIM, N], bf16, tag="a2")
                nc.vector.tensor_tensor(out=a2, in0=t01, in1=T[:, 4, :],
                                        op=ALU.add)
                numy = pacc.tile([F_DIM, N], bf16, tag="numy")
                nc.vector.tensor_tensor(out=numy, in0=a2, in1=t23,
                                        op=ALU.add)
                y0 = pacc.tile([F_DIM, N], f32r, tag="y0")
                nc.vector.tensor_tensor(out=y0, in0=numy,
                                        in1=Rps, op=ALU.mult)

                xps = alloc_sm()
                nc.tensor.matmul(xps, wo, y0, start=True, stop=False)
                nc.tensor.matmul(xps, w0c, r_sb, start=False, stop=True)
                x_sb = pacc.tile([F_DIM, N], f32r, tag="x_sb")
                nc.scalar.activation(x_sb, xps, ACT.Relu)
                mps = alloc_sm()[0:H, :]
                nc.tensor.matmul(mps, headw8, x_sb, start=True, stop=True)
                mg = pacc.tile([H, N], f32, tag="mg")
                nc.scalar.activation(mg, mps, ACT.Exp)

                # gumbel tail in natural layout: transpose exp(m) via PE
                mT = alloc_sm()[:, 0:32].rearrange("p (j c) -> p j c", c=8)
                for ch in range(4):
                    nc.tensor.transpose(
                        mT[:, ch, :], mg[0:8, ch * 128:(ch + 1) * 128],
                        id8.bitcast(f32))
                ez = ptail.tile([128, 4, 5], f32, tag="ez")
                nc.vector.tensor_tensor(
                    out=ez, in0=mT[:, :, 0:5],
                    in1=wrec_all[:, 4 * t:4 * t + 4, :], op=ALU.mult)
                ssum = ptail.tile([128, 4, 1], f32, tag="ssum")
                nc.vector.reduce_sum(ssum, ez, axis=AX.X)
                rg = ptail.tile([128, 4, 1], f32, tag="rg")
                nc.vector.reciprocal_approx_fast(
                    out=rg.rearrange("p j c -> p (j c)"),
                    in_=ssum.rearrange("p j c -> p (j c)"))
                pol = ptail.tile([128, 4, 5], f32, tag="pol")
                nc.gpsimd.tensor_tensor(
                    out=pol, in0=ez, in1=rg.to_broadcast([128, 4, 5]),
                    op=ALU.mult)
                nc.scalar.dma_start(
                    out=out_d[base:base + N, :].rearrange(
                        "(j p) c -> p j c", p=128),
                    in_=pol)

            rep_ctx = (tc.For_i(0, repeat, 1) if repeat > 1
                       else contextlib.nullcontext())
            with rep_ctx:
                pend = deque()
                for t in range(nt):
                    if len(pend) > 1:
                        tt, fr = pend.popleft()
                        emit_back(tt, *fr)
                    pend.append((t, emit_front(t)))
                while pend:
                    tt, fr = pend.popleft()
                    emit_back(tt, *fr)
    nc.compile()
    return nc


LAST_PROFILE = {}


def kernel(_trace=False, **inputs):
    from concourse.bass_utils import run_bass_kernel_spmd

    cblk, z = _consts_from_weights(inputs)
    obs = np.asarray(inputs["obs"], np.float32)
    u = np.asarray(inputs["u"], np.float32)

    nc = build_program(BC, z=z)
    in_maps = []
    for c in range(NCORES):
        obsT, uu = _prep_core_inputs(obs, u, c)
        in_maps.append(dict(obsT=obsT, u=uu,
                            cblk1=np.ascontiguousarray(cblk[:, :5 * F_DIM]),
                            cblk2=np.ascontiguousarray(cblk[:, 5 * F_DIM:])))
    res = run_bass_kernel_spmd(nc, in_maps, list(range(NCORES)), trace=_trace)
    LAST_PROFILE.clear()
    LAST_PROFILE.update(dict(exec_time_ns=res.exec_time_ns))
    out = np.concatenate([res.results[c]["out"] for c in range(NCORES)], axis=0)
    return out
